# revision 23
# baseline (speedup 1.0000x reference)
"""Trainium2 Bass kernel for chunkwise retention (B=8, S=2048, D=512, H=512,
chunk=2, gamma=0.984375, counter=1).

Sharding: data-parallel over batch — core i processes batch element i. No
collectives needed (all compute is batch-independent).

Algorithm — block-parallel reformulation of the 1024-step scan, processed in
PAIRS of 128-position blocks (validated at ~8e-7 rel err in f32):

  qkv = x @ W + b; per micro-chunk t (2 positions):
    out_t = norm(Q_t K_t^T * c * Dsub) V_t + gq * Q_t state_t
    state_{t+1} = gc * state_t + K_t^T V_t           (gc = gq = gamma^2)

  Per pair m (blocks P0/P1, 256 positions, 128 micro-chunks):
    - Q' = Q * qs2 (per-position decay folded in; period 256)
    - PT_a = K(P0) @ Q'(P0..P1)^T  [128x256]  (diag of P0 + cross P0->P1)
    - PT_b = K(P1) @ Q'(P1)^T      [128x128]  (diag of P1)
    - constant masks CDIAG*/CLOW* recover the normalized local 2x2 attention
      and the decay-weighted strictly-lower attention from PT via elementwise
      ops; one row-reduction gives the normalizer.
    - cross-pair history via a running state accumulated in PSUM in a global
      scale (S~ = S / gc^{128 m}; increment row weights gc^{-128(m+1)+127-j}),
      read back once per pair through a scalar-engine scaled copy.

Precision/speed scheme (validated in numpy at ~4e-3 max-rel, gate is 2e-2):
  - the qkv projection runs in fp8e4 (e4m3) DoubleRow perf mode, which packs
    two 128-deep contraction tiles per matmul at 0.5 cycles/row. x and 64*W
    are decomposed on the host into hi + lo fp8 halves (two-level quantization
    captures f32 to ~0.1%); the kernel computes hi*hi plus the cross terms
    (hi*lo + lo*hi) — 6 DoubleRow matmuls per 128x512 psum tile vs 4 full-rate
    fp32r matmuls, a 1.33x PE saving at ~0.15% error. The 1/64 W prescale
    (needed because |W|~0.02 lands in e4m3's subnormal range) is folded into
    the psum->SBUF copy scales.
  - everything downstream (PT, A@V, state update, readout) runs in bf16:
    same PE rate as fp32r for moving>=256, full rate (vs 1/4 for fp32r) for
    the 128-wide PT_b, and 1.0 cyc/row transposes (vs 1.5).
  - PE p-state ramp is pre-warmed with a 4-matmul dummy accumulation group on
    a memset tile while the first DMAs stream in.

kernel(**inputs) is self-contained: constants embedded via inline_tensor,
shapes hardcoded, 8-way batch sharding + host-side fp8 packing + gather done
on host.
"""

import numpy as np
import ml_dtypes

import concourse.bass as bass
import concourse.mybir as mybir
import concourse.tile as tile
from concourse.bass_utils import run_bass_kernel_spmd

F32 = mybir.dt.float32
BF16 = mybir.dt.bfloat16
F8 = mybir.dt.float8e4
AF = mybir.ActivationFunctionType
DR = mybir.MatmulPerfMode.DoubleRow
ALU = mybir.AluOpType

B, S, D, H = 8, 2048, 512, 512
GAMMA = 0.984375
L = 128                  # block positions
NB = S // L              # 16 blocks
NP = NB // 2             # 8 pairs
NKD = D // 128           # 4 k-tiles over d
NKH = H // 128           # 4 k-tiles over h
GC = GAMMA * GAMMA
WS = 64.0                # host W prescale (keeps fp8 W out of subnormals)

_f8 = ml_dtypes.float8_e4m3
_bf = ml_dtypes.bfloat16


def _consts(pad_b: bool):
    g = np.float64(GAMMA)
    gc = g * g
    r1, r2 = 1 + g, 1 + g + g * g
    Dsub = np.array([[1 / np.sqrt(r1), 0.0], [g / np.sqrt(r1), 1 / np.sqrt(r2)]])
    ish = 1.0 / np.sqrt(np.float64(H))
    lp2 = np.arange(2 * L)
    qs2 = gc * gc ** (lp2 // 2)
    r0 = np.arange(L)
    j0 = r0[:, None] // 2
    tA = lp2[None, :] // 2
    CLOWa = np.where(j0 < tA, gc ** (-1.0 - j0), 0.0) * np.ones((L, 2 * L))
    dm = ish * Dsub[(lp2[None, :] % 2), (r0[:, None] % 2)] / qs2[None, :]
    CDIAGa = np.where(j0 == tA, dm, 0.0)
    j1 = 64 + r0[:, None] // 2
    t1 = 64 + (np.arange(L)[None, :] // 2)
    CLOWb = np.where(j1 < t1, gc ** (-1.0 - j1), 0.0) * np.ones((L, L))
    dmb = ish * Dsub[(np.arange(L)[None, :] % 2), (r0[:, None] % 2)] / qs2[None, 128:]
    CDIAGb = np.where(j1 == t1, dmb, 0.0)
    if pad_b:
        # f32r needs a 256-wide moving dim for full rate; garbage columns are
        # masked to zero. bf16 runs full-rate at 128 so the fast path skips it.
        CLOWb = np.concatenate([CLOWb, np.zeros((L, L))], axis=1)
        CDIAGb = np.concatenate([CDIAGb, np.zeros((L, L))], axis=1)
    WST = np.zeros((L, NB))
    for I in range(NB):
        m = I // 2
        WST[:, I] = gc ** (-128.0 * (m + 1)) * gc ** (127.0 - (64.0 * (I % 2) + r0 // 2))
    QSROW = np.tile(qs2[None, :], (128, 2))
    IDN = np.eye(128)
    return [a.astype(np.float32) for a in
            (CLOWa, CDIAGa, CLOWb, CDIAGb, WST, QSROW, IDN)]


def _split_waits(nc):
    """Hoist attached sync waits onto standalone EventSemaphore carriers.

    The walrus build in this container supports only one sync-wait command per
    instruction ("Too many sync wait commands") while Tile attaches all waits
    to the consuming instruction. A carrier EventSemaphore per wait, on the
    same engine right before the instruction, is semantically identical.
    """
    for bb in nc.main_func.blocks:
        insts = list(bb.instructions)
        out = []
        for ins in insts:
            si = getattr(ins, "sync_info", None)
            if si is not None and si.on_wait and type(ins).__name__ != "InstEventSemaphore":
                for k, w in enumerate(list(si.on_wait)):
                    ev = mybir.InstEventSemaphore(name=f"{ins.name}-sw{k}", ins=[], outs=[])
                    ev.engine = ins.engine
                    ev.sync_info = mybir.SyncInfo(on_wait=[w], on_update=[])
                    nc.register_instruction(ev)
                    out.append(ev)
                ins.sync_info = mybir.SyncInfo(on_wait=[], on_update=list(si.on_update))
            out.append(ins)
        bb.instructions[:] = out
    return nc


# fp8 phase-A load schedule, all on the SP HWDGE queue so the serial DMA
# engine processes transfers in exactly this order: "w" = one 128-col chunk of
# the QK weights (chunk-major, contiguous in DRAM so the descriptor overhead
# stays low), "x" = one 512-seq chunk of x, "v" = the V-column weights, and
# constants placed at the latest spot before their first consumer.
DMA_PLAN8 = [("xh", 0), ("w", 0), ("w", 1), ("w", 2), ("xh", 1),
             ("c", "qsrow"), ("w", 3), ("w", 4), ("w", 5), ("w", 6), ("w", 7),
             ("x", 1), ("c", "wst"), ("c", "idn"), ("v", 0), ("x", 2),
             ("c", "clowa"), ("c", "cdiaga"), ("x", 3), ("c", "clowb"),
             ("c", "cdiagb")]
PROJ_TILES = [(0, 512), (512, 1024), (1024, 1536), (1536, 2048)]


def build_nc_fp8():
    CLOWa, CDIAGa, CLOWb, CDIAGb, WST, QSROW, IDN = _consts(pad_b=False)
    QSROW64 = (QSROW / WS).astype(np.float32)
    nc = bass.Bass()
    xcr_d = nc.declare_dram_parameter("xcr", [4, 128, NKD, 2, 512], F8,
                                      isOutput=False)
    # duplicate of x's first 256 seq columns, contiguous, so the very first
    # load is byte-bound (the strided half-chunk slice is descriptor-bound)
    x0a_d = nc.declare_dram_parameter("x0a", [128, NKD, 2, 256], F8,
                                      isOutput=False)
    wcr_d = nc.declare_dram_parameter("wcr", [8, 128, NKD, 2, 128], F8,
                                      isOutput=False)
    wv_d = nc.declare_dram_parameter("wv", [128, NKD, 2, H], F8, isOutput=False)
    out_d = nc.declare_dram_parameter("out", [S, H], F32, isOutput=True)

    clowa_d = nc.inline_tensor(CLOWa, "clowa")
    cdiaga_d = nc.inline_tensor(CDIAGa, "cdiaga")
    clowb_d = nc.inline_tensor(CLOWb, "clowb")
    cdiagb_d = nc.inline_tensor(CDIAGb, "cdiagb")
    wst_d = nc.inline_tensor(WST, "wst")
    qsrow_d = nc.inline_tensor(QSROW64, "qsrow")
    idn_d = nc.inline_tensor(IDN.astype(_bf), "idn")

    with tile.TileContext(nc) as tc:
        with (
            tc.tile_pool(name="singles", bufs=1) as singles,
            tc.tile_pool(name="bigbufs", bufs=1) as bigbufs,
            tc.tile_pool(name="spsum", bufs=1, space="PSUM") as spsum,
        ):

            # ---- constants (loaded via DMA_PLAN8 on the SP queue) ----
            qsrow = singles.tile([128, 512], F32)
            wst = singles.tile([128, NB], F32)
            clowa = singles.tile([L, 2 * L], F32)
            cdiaga = singles.tile([L, 2 * L], F32)
            clowb = singles.tile([L, L], F32)
            cdiagb = singles.tile([L, L], F32)
            idn = singles.tile([128, 128], BF16)
            const_map = {"qsrow": (qsrow, qsrow_d), "wst": (wst, wst_d),
                         "clowa": (clowa, clowa_d), "cdiaga": (cdiaga, cdiaga_d),
                         "clowb": (clowb, clowb_d), "cdiagb": (cdiagb, cdiagb_d),
                         "idn": (idn, idn_d)}

            # ---- persistent SBUF operands (all bf16) ----
            qTb = bigbufs.tile([128, NKH, S], BF16, tag="qT")
            kTb = bigbufs.tile([128, NKH, S], BF16, tag="kT")
            vb = bigbufs.tile([128, NB, H], BF16, tag="v")
            kwb = bigbufs.tile([128, NB, H], BF16, tag="kw")
            stil = [spsum.tile([128, H], F32, tag=f"s{t}", name=f"stil{t}")
                    for t in range(NKH)]

            # ---- phase A: fp8 DoubleRow projections + K transposes ----
            with (
                tc.tile_pool(name="xw", bufs=1) as xw,
                tc.tile_pool(name="ppsum", bufs=4, space="PSUM") as rot,
            ):
                # PE warm-up: ramp the p-state while the first DMAs land.
                # 6 dummy 512-row matmuls keep the engine continuously busy
                # until the first projection matmul is ready, so real work
                # starts at the full 2.4 GHz p-state.
                wu = xw.tile([128, 512], BF16, tag="wu", name="wu")
                nc.gpsimd.memset(wu, 0.25)
                wupp = rot.tile([128, 512], F32, tag="pp", name="wupp")
                for i in range(6):
                    nc.tensor.matmul(wupp, lhsT=wu[:, 0:128], rhs=wu,
                                     start=(i == 0), stop=(i == 5),
                                     skip_group_check=True)

                xs = xw.tile([128, 4, NKD, 2, 512], F8, tag="xcr")
                ws = xw.tile([128, 8, NKD, 2, 128], F8, tag="wcr")
                wv = xw.tile([128, NKD, 2, H], F8, tag="wv")
                for kind, c in DMA_PLAN8:
                    if kind == "w":
                        nc.sync.dma_start(out=ws[:, c], in_=wcr_d[c])
                    elif kind == "xh":
                        if c == 0:
                            nc.sync.dma_start(out=xs[:, 0, :, :, 0:256],
                                              in_=x0a_d[:, :, :, :])
                        else:
                            nc.sync.dma_start(out=xs[:, 0, :, :, 256:512],
                                              in_=xcr_d[0][:, :, :, 256:512])
                    elif kind == "x":
                        nc.sync.dma_start(out=xs[:, c], in_=xcr_d[c])
                    elif kind == "v":
                        nc.sync.dma_start(out=wv, in_=wv_d[:, :, :, :])
                    else:
                        t, d = const_map[c]
                        nc.sync.dma_start(out=t, in_=d[:, :])

                # Q'^T and K^T tiles: [3h-tile, seq-tile]. The first seq
                # chunk runs as two 256-wide half-groups so matmuls start as
                # soon as the first half of x0 lands.
                for nt in range(4):
                    c0 = nt * 512
                    halves = ((0, 256), (256, 512)) if nt == 0 else ((0, 512),)
                    for mt in range(8):
                        pp = rot.tile([128, 512], F32, tag="pp", name="ppqk")
                        for h0, h1 in halves:
                            for j, kp in enumerate((0, 2)):
                                nc.tensor.matmul(
                                    pp[:, h0:h1],
                                    lhsT=ws[:, mt, kp:kp + 2, 0, :],
                                    rhs=xs[:, nt, kp:kp + 2, 1, h0:h1],
                                    start=(j == 0), stop=False,
                                    perf_mode=DR, skip_group_check=True)
                            for kt in range(NKD):
                                nc.tensor.matmul(
                                    pp[:, h0:h1], lhsT=ws[:, mt, kt, :, :],
                                    rhs=xs[:, nt, kt, :, h0:h1],
                                    start=False, stop=(kt == NKD - 1),
                                    perf_mode=DR, skip_group_check=True)
                        if mt < 4:
                            nc.vector.tensor_mul(
                                qTb[:, mt, c0:c0 + 512], pp, qsrow)
                        else:
                            nc.scalar.activation(
                                kTb[:, mt - 4, c0:c0 + 512], pp, AF.Copy,
                                scale=1.0 / WS)

                # V rows: [seq-block, h] — x stationary, W_v moving
                for I in range(NB):
                    ch, b = I // 4, I % 4
                    bcols = slice(b * 128, (b + 1) * 128)
                    pp = rot.tile([128, 512], F32, tag="pp", name="ppv")
                    for j, kp in enumerate((0, 2)):
                        nc.tensor.matmul(
                            pp, lhsT=xs[:, ch, kp:kp + 2, 1, bcols],
                            rhs=wv[:, kp:kp + 2, 0, :],
                            start=(j == 0), stop=False,
                            perf_mode=DR, skip_group_check=True)
                    for kt in range(NKD):
                        nc.tensor.matmul(
                            pp, lhsT=xs[:, ch, kt, :, bcols],
                            rhs=wv[:, kt, :, :],
                            start=False, stop=(kt == NKD - 1),
                            perf_mode=DR, skip_group_check=True)
                    nc.vector.tensor_scalar_mul(vb[:, I, :], pp, 1.0 / WS)

                # weighted K rows via PE transpose of K^T + scaled copy
                for I in range(NB):
                    pt = rot.tile([128, 512], BF16, tag="pp", name="ppt")
                    for kt in range(NKH):
                        nc.tensor.matmul(
                            pt[:, kt * 128:(kt + 1) * 128],
                            lhsT=kTb[:, kt, I * 128:(I + 1) * 128], rhs=idn,
                            is_transpose=True, start=True, stop=True,
                            skip_group_check=True)
                    nc.scalar.activation(
                        kwb[:, I, :], pt, AF.Copy, scale=wst[:, I:I + 1])

            # ---- phase B: per-pair attention + state scan ----
            with (
                tc.tile_pool(name="blk", bufs=4) as blk,
                tc.tile_pool(name="bpsum", bufs=2, space="PSUM") as bpsum,
                tc.tile_pool(name="srd", bufs=3) as srd,
            ):
                def emit_pt(m):
                    s0 = bass.ts(2 * m, L)
                    s1 = bass.ts(2 * m + 1, L)
                    sq = bass.ds(2 * m * L, 2 * L)
                    ptab = bpsum.tile([L, 3 * L], F32, tag="pt", name="ptab")
                    pta = ptab[:, 0:2 * L]
                    ptb = ptab[:, 2 * L:3 * L]
                    for kt in range(NKH):
                        nc.tensor.matmul(pta, lhsT=kTb[:, kt, s0],
                                         rhs=qTb[:, kt, sq],
                                         start=(kt == 0), stop=(kt == NKH - 1),
                                         skip_group_check=True)
                    for kt in range(NKH):
                        nc.tensor.matmul(ptb, lhsT=kTb[:, kt, s1],
                                         rhs=qTb[:, kt, s1],
                                         start=(kt == 0), stop=(kt == NKH - 1),
                                         skip_group_check=True)
                    return pta, ptb

                def emit_chain(m, pta, ptb):
                    aca = blk.tile([L, 2 * L], BF16, tag="aca", name="aca")
                    acb = blk.tile([L, L], BF16, tag="acb", name="acb")
                    for (pt, cd, cl, ac, w) in (
                            (pta, cdiaga, clowa, aca, 2 * L),
                            (ptb, cdiagb, clowb, acb, L)):
                        scratch = blk.tile([L, 2 * L], F32, tag="scratch",
                                           name="scratch")
                        nc.vector.tensor_mul(scratch[:, :w], pt[:, :w], cd[:, :w])
                        rs = blk.tile([L, 1], F32, tag="rs", name="rs")
                        nc.vector.reduce_sum(out=rs, in_=scratch[:, :w],
                                             axis=mybir.AxisListType.X)
                        # pcl is independent of the reduce -> off critical path
                        pcl = blk.tile([L, 2 * L], F32, tag="pcl", name="pcl")
                        nc.vector.tensor_mul(pcl[:, :w], pt[:, :w], cl[:, :w])
                        rcp = blk.tile([L, 1], F32, tag="rcp", name="rcp")
                        nc.scalar.activation(rcp, rs, AF.Abs)
                        nc.vector.tensor_scalar_max(rcp, rcp, 1.0)
                        nc.vector.reciprocal(rcp, rcp)
                        # ac = (pt*cd)*rcp + pt*cl fused
                        nc.vector.scalar_tensor_tensor(
                            ac[:, :w], scratch[:, :w], rcp, pcl[:, :w],
                            ALU.mult, ALU.add)
                    return aca, acb

                def emit_rb(m):
                    # state readback for pair m, issued right after pair m-1's
                    # state update so it overlaps pair m-1's output matmuls;
                    # kt3 on DVE, rest on Act
                    rb = float(GC ** (128.0 * m))
                    ssbs = []
                    for kt in range(NKH):
                        ssb = srd.tile([128, H], BF16, tag=f"ssb{kt}",
                                       name=f"ssb{kt}")
                        if kt == 3:
                            nc.vector.tensor_scalar_mul(ssb, stil[kt], rb)
                        else:
                            nc.scalar.activation(ssb, stil[kt], AF.Copy,
                                                 scale=rb)
                        ssbs.append(ssb)
                    return ssbs

                def emit_ops(m, aca, acb, ssbs):
                    s0 = bass.ts(2 * m, L)
                    s1 = bass.ts(2 * m + 1, L)
                    last = (m == NP - 1)
                    osb = blk.tile([L, 2, H], F32, tag="osb", name="osb")
                    # out(P0) = Aca[:, :128]^T V(P0) (+ Q'(P0) S)
                    op0 = bpsum.tile([L, H], F32, tag="out", name="op0")
                    nc.tensor.matmul(op0, lhsT=aca[:, 0:L], rhs=vb[:, 2 * m, :],
                                     start=True, stop=(m == 0))
                    if m > 0:
                        for kt in range(NKH):
                            nc.tensor.matmul(op0, lhsT=qTb[:, kt, s0],
                                             rhs=ssbs[kt],
                                             start=False, stop=(kt == NKH - 1))
                    # copies split Act/DVE so each psum bank frees in ~400ns;
                    # the last pair keeps everything on DVE + the SP queue so
                    # no cross-engine wait sits in front of the final DMAs
                    nc.scalar.activation(osb[:, 0, :], op0, AF.Copy)
                    if last:
                        nc.sync.dma_start(out=out_d[s0, :], in_=osb[:, 0, :])

                    # out(P1) = Aca[:, 128:]^T V(P0) + Acb^T V(P1) (+ Q'(P1) S)
                    op1 = bpsum.tile([L, H], F32, tag="out", name="op1")
                    nc.tensor.matmul(op1, lhsT=aca[:, L:2 * L],
                                     rhs=vb[:, 2 * m, :],
                                     start=True, stop=False)
                    nc.tensor.matmul(op1, lhsT=acb, rhs=vb[:, 2 * m + 1, :],
                                     start=False, stop=(m == 0))
                    if m > 0:
                        for kt in range(NKH):
                            nc.tensor.matmul(op1, lhsT=qTb[:, kt, s1],
                                             rhs=ssbs[kt],
                                             start=False, stop=(kt == NKH - 1))
                    if last:
                        nc.vector.tensor_copy(osb[:, 1, :], op1)
                        nc.sync.dma_start(out=out_d[s1, :], in_=osb[:, 1, :])
                    else:
                        nc.scalar.activation(osb[:, 1, :], op1, AF.Copy)
                        # one 256-row DMA per pair halves the out-DMA
                        # dispatches; rearrange makes the DRAM side iterate
                        # (pos, block, col) to match the SBUF tile layout
                        out_v = out_d.rearrange("(b l p) c -> p (b l) c",
                                                l=2, p=L)
                        nc.sync.dma_start(out=out_v[:, 2 * m:2 * m + 2, :],
                                          in_=osb)

                    # state update; the last pair's state is never read
                    if m < NP - 1:
                        for blki in (2 * m, 2 * m + 1):
                            for ht in range(NKH):
                                nc.tensor.matmul(
                                    stil[ht],
                                    lhsT=kwb[:, blki, ht * 128:(ht + 1) * 128],
                                    rhs=vb[:, blki, :],
                                    start=(m == 0 and blki == 0),
                                    stop=(m == NP - 2 and blki == 2 * m + 1),
                                    skip_group_check=True)
                        return emit_rb(m + 1)
                    return None

                # software pipeline: PT(m+1) is issued before ops(m) so the
                # tensor engine never stalls on pair m's DVE mask chain
                prev = None
                ssbs = None
                for m in range(NP):
                    pt = emit_pt(m)
                    ch = emit_chain(m, *pt)
                    if prev is not None:
                        ssbs = emit_ops(m - 1, *prev, ssbs)
                    prev = ch
                emit_ops(NP - 1, *prev, ssbs)
    return _split_waits(nc)


_NC_CACHE = {}

# test-harness knobs (the graded path leaves these at defaults)
TRACE = False
LAST_RESULT = None


def build_nc(has_bias: bool):
    return build_nc_fp8()


def _get_nc(key):
    if key not in _NC_CACHE:
        _NC_CACHE[key] = build_nc_fp8() if key == "fp8" else build_nc_f32r(True)
    return _NC_CACHE[key]


def _pack_levels(a):
    """[512, N] f32 -> hi/lo fp8 packed [128, NKD, 2, N] with lvl0=lo, lvl1=hi
    for x; the W packer flips levels (lvl0=hi, lvl1=lo)."""
    hi = a.astype(_f8)
    lo = (a - hi.astype(np.float32)).astype(_f8)
    return hi, lo


def kernel(x, W, b):
    global LAST_RESULT
    x = np.ascontiguousarray(x, dtype=np.float32)
    W = np.ascontiguousarray(W, dtype=np.float32)
    b = np.ascontiguousarray(b, dtype=np.float32)
    has_bias = bool(np.any(b))
    if has_bias:
        nc = _get_nc("f32r")
        in_maps = [
            {"xT": np.ascontiguousarray(x[i].T), "W": W, "b": b.reshape(1, 3 * H)}
            for i in range(B)
        ]
        res = run_bass_kernel_spmd(nc, in_maps, list(range(B)), trace=TRACE)
        LAST_RESULT = res
        return np.stack([res.results[i]["out"] for i in range(B)], axis=0)

    nc = _get_nc("fp8")
    Whi, Wlo = _pack_levels(W * WS)
    # QK weight chunks: [chunk, part, kt, lvl, col] with lvl0=hi, lvl1=lo
    wcr = np.empty((8, 128, NKD, 2, 128), _f8)
    wv = np.empty((128, NKD, 2, H), _f8)
    for lvl, Wq in ((0, Whi), (1, Wlo)):
        r = Wq.reshape(NKD, 128, 3 * H).transpose(1, 0, 2)  # [part, kt, col]
        for c in range(8):
            wcr[c, :, :, lvl, :] = r[:, :, c * 128:(c + 1) * 128]
        wv[:, :, lvl, :] = r[:, :, 2 * H:]
    in_maps = []
    for i in range(B):
        xT = np.ascontiguousarray(x[i].T)
        xhi, xlo = _pack_levels(xT)
        # x chunks: [chunk, part, kt, lvl, seq] with lvl0=lo, lvl1=hi
        xcr = np.empty((4, 128, NKD, 2, 512), _f8)
        for lvl, xq in ((0, xlo), (1, xhi)):
            r = xq.reshape(NKD, 128, S).transpose(1, 0, 2)
            for c in range(4):
                xcr[c, :, :, lvl, :] = r[:, :, c * 512:(c + 1) * 512]
        x0a = np.ascontiguousarray(xcr[0, :, :, :, 0:256])
        in_maps.append({"xcr": xcr, "wcr": wcr, "wv": wv, "x0a": x0a})
    res = run_bass_kernel_spmd(nc, in_maps, list(range(B)), trace=TRACE)
    LAST_RESULT = res
    return np.stack([res.results[i]["out"] for i in range(B)], axis=0)


# ---------------------------------------------------------------------------
# f32r fallback (bias path; mirrors the validated baseline kernel)
# ---------------------------------------------------------------------------

DMA_PLAN = [("w", 0, 512), ("x", 0, 512), ("w", 512, 1024), ("x", 512, 1024),
            ("x", 1024, 1536), ("x", 1536, 2048), ("w", 1024, 1536)]


def build_nc_f32r(has_bias: bool, mmdt=mybir.dt.float32r):
    CLOWa, CDIAGa, CLOWb, CDIAGb, WST, QSROW, IDN = _consts(pad_b=True)
    QSROW = np.tile(QSROW[:1], (128, 1)).astype(np.float32)
    nc = bass.Bass()
    xT_d = nc.declare_dram_parameter("xT", [D, S], mmdt, isOutput=False)
    w_d = nc.declare_dram_parameter("W", [D, 3 * H], mmdt, isOutput=False)
    b_d = nc.declare_dram_parameter("b", [1, 3 * H], mmdt, isOutput=False)
    out_d = nc.declare_dram_parameter("out", [S, H], F32, isOutput=True)

    clowa_d = nc.inline_tensor(CLOWa, "clowa")
    cdiaga_d = nc.inline_tensor(CDIAGa, "cdiaga")
    clowb_d = nc.inline_tensor(CLOWb, "clowb")
    cdiagb_d = nc.inline_tensor(CDIAGb, "cdiagb")
    wst_d = nc.inline_tensor(WST, "wst")
    qsrow_d = nc.inline_tensor(QSROW, "qsrow")
    idn_d = nc.inline_tensor(IDN, "idn")

    with tile.TileContext(nc) as tc:
        with (
            tc.tile_pool(name="singles", bufs=1) as singles,
            tc.tile_pool(name="bigbufs", bufs=1) as bigbufs,
            tc.tile_pool(name="spsum", bufs=1, space="PSUM") as spsum,
        ):
            clowa = singles.tile([L, 2 * L], F32)
            nc.gpsimd.dma_start(out=clowa, in_=clowa_d[:, :])
            cdiaga = singles.tile([L, 2 * L], F32)
            nc.gpsimd.dma_start(out=cdiaga, in_=cdiaga_d[:, :])
            clowb = singles.tile([L, 2 * L], F32)
            nc.gpsimd.dma_start(out=clowb, in_=clowb_d[:, :])
            cdiagb = singles.tile([L, 2 * L], F32)
            nc.gpsimd.dma_start(out=cdiagb, in_=cdiagb_d[:, :])
            wst = singles.tile([128, NB], F32)
            nc.gpsimd.dma_start(out=wst, in_=wst_d[:, :])
            qsrow = singles.tile([128, 512], F32)
            nc.gpsimd.dma_start(out=qsrow, in_=qsrow_d[:, :])
            idn = singles.tile([128, 128], mmdt)
            nc.gpsimd.dma_start(out=idn, in_=idn_d[:, :])

            qT = bigbufs.tile([128, NKH, S], mmdt, tag="qT")
            kT = bigbufs.tile([128, NKH, S], mmdt, tag="kT")
            v_all = bigbufs.tile([128, NB, H], mmdt, tag="v")
            kw_all = bigbufs.tile([128, NB, H], mmdt, tag="kw")
            stil = [spsum.tile([128, H], F32, tag=f"s{t}", name=f"stil{t}")
                    for t in range(NKH)]

            with (
                tc.tile_pool(name="xw", bufs=1) as xw,
                tc.tile_pool(name="ppsum", bufs=4, space="PSUM") as ppsum,
            ):
                xT_r = xT_d.rearrange("(k p) s -> p k s", p=128)
                w_r = w_d.rearrange("(k p) m -> p k m", p=128)
                xTs = xw.tile([128, NKD, S], mmdt, tag="xT")
                ws = xw.tile([128, NKD, 3 * H], mmdt, tag="w")

                def ldw(c0, c1):
                    for kt in range(NKD):
                        nc.sync.dma_start(out=ws[:, kt, c0:c1], in_=w_r[:, kt, c0:c1])

                def ldx(c0, c1):
                    for kt in range(NKD):
                        nc.sync.dma_start(out=xTs[:, kt, c0:c1], in_=xT_r[:, kt, c0:c1])

                for (kind, c0, c1) in DMA_PLAN:
                    (ldw if kind == "w" else ldx)(c0, c1)
                if has_bias:
                    brow = xw.tile([1, 3 * H], mmdt, tag="b")
                    nc.sync.dma_start(out=brow, in_=b_d[:, :])
                    ones = xw.tile([1, S], mmdt, tag="ones")
                    nc.vector.memset(ones, 1.0)

                for nt, (c0, c1) in enumerate(PROJ_TILES):
                    w_ = c1 - c0
                    for mt in range(8):
                        pp = ppsum.tile([128, 512], F32, tag="pp")
                        for kt in range(NKD):
                            nc.tensor.matmul(
                                pp[:, :w_], lhsT=ws[:, kt, mt * 128:(mt + 1) * 128],
                                rhs=xTs[:, kt, c0:c1],
                                start=(kt == 0), stop=(kt == NKD - 1 and not has_bias),
                                skip_group_check=True)
                        if has_bias:
                            nc.tensor.matmul(
                                pp[:, :w_], lhsT=brow[:, mt * 128:(mt + 1) * 128],
                                rhs=ones[:, c0:c1],
                                start=False, stop=True, skip_group_check=True)
                        if mt < 4:
                            nc.vector.tensor_mul(
                                qT[:, mt, c0:c1], pp[:, :w_], qsrow[:, :w_])
                        else:
                            nc.scalar.activation(
                                kT[:, mt - 4, c0:c1], pp[:, :w_], AF.Copy)

                for I in range(NB):
                    pp = ppsum.tile([128, 512], F32, tag="pp")
                    for kt in range(NKD):
                        nc.tensor.matmul(
                            pp, lhsT=xTs[:, kt, I * 128:(I + 1) * 128],
                            rhs=ws[:, kt, 2 * H:3 * H],
                            start=(kt == 0), stop=(kt == NKD - 1 and not has_bias))
                    if has_bias:
                        nc.tensor.matmul(
                            pp, lhsT=ones[:, I * 128:(I + 1) * 128],
                            rhs=brow[:, 2 * H:3 * H], start=False, stop=True)
                    nc.scalar.activation(v_all[:, I, :], pp, AF.Copy)

                for I in range(NB):
                    pp = ppsum.tile([128, 512], mmdt, tag="pp")
                    for kt in range(NKH):
                        nc.tensor.matmul(
                            pp[:, kt * 128:(kt + 1) * 128],
                            lhsT=kT[:, kt, I * 128:(I + 1) * 128], rhs=idn,
                            is_transpose=True, start=True, stop=True,
                            skip_group_check=True)
                    nc.scalar.activation(
                        kw_all[:, I, :], pp, AF.Copy, scale=wst[:, I:I + 1])

            with (
                tc.tile_pool(name="blk", bufs=4) as blk,
                tc.tile_pool(name="bpsum", bufs=2, space="PSUM") as bpsum,
                tc.tile_pool(name="srd", bufs=3) as srd,
            ):
                for m in range(NP):
                    s0 = bass.ts(2 * m, L)
                    s1 = bass.ts(2 * m + 1, L)
                    sq = bass.ds(2 * m * L, 2 * L)
                    wb = 2 * L if m < NP - 1 else L
                    sb = bass.ds((2 * m + 1) * L, wb)
                    ptab = bpsum.tile([L, 4 * L], F32, tag="pt")
                    pta = ptab[:, 0:2 * L]
                    ptb = ptab[:, 2 * L:3 * L]
                    for kt in range(NKH):
                        nc.tensor.matmul(pta, lhsT=kT[:, kt, s0], rhs=qT[:, kt, sq],
                                         start=(kt == 0), stop=(kt == NKH - 1),
                                         skip_group_check=True)
                    for kt in range(NKH):
                        nc.tensor.matmul(ptab[:, 2 * L:2 * L + wb],
                                         lhsT=kT[:, kt, s1], rhs=qT[:, kt, sb],
                                         start=(kt == 0), stop=(kt == NKH - 1),
                                         skip_group_check=True)

                    aca = blk.tile([L, 2 * L], mmdt, tag="aca")
                    acb = blk.tile([L, L], mmdt, tag="acb")
                    for (pt, cd, cl, ac, w) in ((pta, cdiaga, clowa, aca, 2 * L),
                                                (ptb, cdiagb, clowb, acb, L)):
                        scratch = blk.tile([L, 2 * L], F32, tag="scratch")
                        nc.vector.tensor_mul(scratch[:, :w], pt[:, :w], cd[:, :w])
                        rs = blk.tile([L, 1], F32, tag="rs")
                        nc.vector.reduce_sum(out=rs, in_=scratch[:, :w],
                                             axis=mybir.AxisListType.X)
                        rcp = blk.tile([L, 1], F32, tag="rcp")
                        nc.scalar.activation(rcp, rs, AF.Abs)
                        nc.vector.tensor_scalar_max(rcp, rcp, 1.0)
                        nc.vector.reciprocal(rcp, rcp)
                        pcl = blk.tile([L, 2 * L], F32, tag="pcl")
                        nc.vector.tensor_mul(pcl[:, :w], pt[:, :w], cl[:, :w])
                        nc.vector.scalar_tensor_tensor(
                            ac[:, :w], scratch[:, :w], rcp, pcl[:, :w],
                            ALU.mult, ALU.add)

                    if m > 0:
                        rb = float(GC ** (128.0 * m))
                        ssbs = []
                        for kt in range(NKH):
                            ssb = srd.tile([128, H], mmdt, tag=f"ssb{kt}",
                                           name=f"ssb{kt}")
                            nc.scalar.activation(ssb, stil[kt], AF.Copy, scale=rb)
                            ssbs.append(ssb)

                    op0 = bpsum.tile([L, H], F32, tag="out")
                    nc.tensor.matmul(op0, lhsT=aca[:, 0:L], rhs=v_all[:, 2 * m, :],
                                     start=True, stop=(m == 0))
                    if m > 0:
                        for kt in range(NKH):
                            nc.tensor.matmul(op0, lhsT=qT[:, kt, s0], rhs=ssbs[kt],
                                             start=False, stop=(kt == NKH - 1))
                    osb0 = blk.tile([L, H], F32, tag="osb0")
                    nc.vector.tensor_copy(osb0, op0)
                    nc.sync.dma_start(out=out_d[s0, :], in_=osb0)

                    op1 = bpsum.tile([L, H], F32, tag="out")
                    nc.tensor.matmul(op1, lhsT=aca[:, L:2 * L], rhs=v_all[:, 2 * m, :],
                                     start=True, stop=False)
                    nc.tensor.matmul(op1, lhsT=acb, rhs=v_all[:, 2 * m + 1, :],
                                     start=False, stop=(m == 0))
                    if m > 0:
                        for kt in range(NKH):
                            nc.tensor.matmul(op1, lhsT=qT[:, kt, s1], rhs=ssbs[kt],
                                             start=False, stop=(kt == NKH - 1))
                    osb1 = blk.tile([L, H], F32, tag="osb1")
                    nc.vector.tensor_copy(osb1, op1)
                    nc.sync.dma_start(out=out_d[s1, :], in_=osb1)

                    for blki in (2 * m, 2 * m + 1):
                        for ht in range(NKH):
                            nc.tensor.matmul(
                                stil[ht],
                                lhsT=kw_all[:, blki, ht * 128:(ht + 1) * 128],
                                rhs=v_all[:, blki, :],
                                start=(m == 0 and blki == 0), stop=(m == NP - 1),
                                skip_group_check=True)
    return _split_waits(nc)


if __name__ == "__main__":
    xs = np.random.randn(B, S, D).astype(np.float32)
    Ws = (np.random.randn(D, 3 * H) * 0.02).astype(np.float32)
    bs = np.zeros(3 * H, np.float32)
    out = kernel(xs, Ws, bs)
    print(out.shape, out.dtype)


# revision 25
# speedup vs baseline: 1.0724x; 1.0724x over previous
"""Trainium2 Bass kernel for chunkwise retention (B=8, S=2048, D=512, H=512,
chunk=2, gamma=0.984375, counter=1).

Sharding: data-parallel over batch — core i processes batch element i. No
collectives needed (all compute is batch-independent).

Algorithm — block-parallel reformulation of the 1024-step scan, processed in
PAIRS of 128-position blocks (validated at ~8e-7 rel err in f32):

  qkv = x @ W + b; per micro-chunk t (2 positions):
    out_t = norm(Q_t K_t^T * c * Dsub) V_t + gq * Q_t state_t
    state_{t+1} = gc * state_t + K_t^T V_t           (gc = gq = gamma^2)

  Per pair m (blocks P0/P1, 256 positions, 128 micro-chunks):
    - Q' = Q * qs2 (per-position decay folded in; period 256)
    - PT_a = K(P0) @ Q'(P0..P1)^T  [128x256]  (diag of P0 + cross P0->P1)
    - PT_b = K(P1) @ Q'(P1)^T      [128x128]  (diag of P1)
    - constant masks CDIAG*/CLOW* recover the normalized local 2x2 attention
      and the decay-weighted strictly-lower attention from PT via elementwise
      ops; one row-reduction gives the normalizer.
    - cross-pair history via a running state accumulated in PSUM in a global
      scale (S~ = S / gc^{128 m}; increment row weights gc^{-128(m+1)+127-j}),
      read back once per pair through a scalar-engine scaled copy.

Precision/speed scheme (validated in numpy at ~4e-3 max-rel, gate is 2e-2):
  - the qkv projection runs in fp8e4 (e4m3) DoubleRow perf mode, which packs
    two 128-deep contraction tiles per matmul at 0.5 cycles/row. x and 64*W
    are decomposed on the host into hi + lo fp8 halves (two-level quantization
    captures f32 to ~0.1%); the kernel computes hi*hi plus the cross terms
    (hi*lo + lo*hi) — 6 DoubleRow matmuls per 128x512 psum tile vs 4 full-rate
    fp32r matmuls, a 1.33x PE saving at ~0.15% error. The 1/64 W prescale
    (needed because |W|~0.02 lands in e4m3's subnormal range) is folded into
    the psum->SBUF copy scales.
  - everything downstream (PT, A@V, state update, readout) runs in bf16:
    same PE rate as fp32r for moving>=256, full rate (vs 1/4 for fp32r) for
    the 128-wide PT_b, and 1.0 cyc/row transposes (vs 1.5).
  - PE p-state ramp is pre-warmed with a 4-matmul dummy accumulation group on
    a memset tile while the first DMAs stream in.

kernel(**inputs) is self-contained: constants embedded via inline_tensor,
shapes hardcoded, 8-way batch sharding + host-side fp8 packing + gather done
on host.
"""

import numpy as np
import ml_dtypes

import concourse.bass as bass
import concourse.mybir as mybir
import concourse.tile as tile
from concourse.bass_utils import run_bass_kernel_spmd

F32 = mybir.dt.float32
BF16 = mybir.dt.bfloat16
F8 = mybir.dt.float8e4
AF = mybir.ActivationFunctionType
DR = mybir.MatmulPerfMode.DoubleRow
ALU = mybir.AluOpType

B, S, D, H = 8, 2048, 512, 512
GAMMA = 0.984375
L = 128                  # block positions
NB = S // L              # 16 blocks
NP = NB // 2             # 8 pairs
NKD = D // 128           # 4 k-tiles over d
NKH = H // 128           # 4 k-tiles over h
GC = GAMMA * GAMMA
WS = 64.0                # host W prescale (keeps fp8 W out of subnormals)

_f8 = ml_dtypes.float8_e4m3
_bf = ml_dtypes.bfloat16


def _consts(pad_b: bool):
    g = np.float64(GAMMA)
    gc = g * g
    r1, r2 = 1 + g, 1 + g + g * g
    Dsub = np.array([[1 / np.sqrt(r1), 0.0], [g / np.sqrt(r1), 1 / np.sqrt(r2)]])
    ish = 1.0 / np.sqrt(np.float64(H))
    lp2 = np.arange(2 * L)
    qs2 = gc * gc ** (lp2 // 2)
    r0 = np.arange(L)
    j0 = r0[:, None] // 2
    tA = lp2[None, :] // 2
    CLOWa = np.where(j0 < tA, gc ** (-1.0 - j0), 0.0) * np.ones((L, 2 * L))
    dm = ish * Dsub[(lp2[None, :] % 2), (r0[:, None] % 2)] / qs2[None, :]
    CDIAGa = np.where(j0 == tA, dm, 0.0)
    j1 = 64 + r0[:, None] // 2
    t1 = 64 + (np.arange(L)[None, :] // 2)
    CLOWb = np.where(j1 < t1, gc ** (-1.0 - j1), 0.0) * np.ones((L, L))
    dmb = ish * Dsub[(np.arange(L)[None, :] % 2), (r0[:, None] % 2)] / qs2[None, 128:]
    CDIAGb = np.where(j1 == t1, dmb, 0.0)
    if pad_b:
        # f32r needs a 256-wide moving dim for full rate; garbage columns are
        # masked to zero. bf16 runs full-rate at 128 so the fast path skips it.
        CLOWb = np.concatenate([CLOWb, np.zeros((L, L))], axis=1)
        CDIAGb = np.concatenate([CDIAGb, np.zeros((L, L))], axis=1)
    WST = np.zeros((L, NB))
    for I in range(NB):
        m = I // 2
        WST[:, I] = gc ** (-128.0 * (m + 1)) * gc ** (127.0 - (64.0 * (I % 2) + r0 // 2))
    QSROW = np.tile(qs2[None, :], (128, 2))
    IDN = np.eye(128)
    return [a.astype(np.float32) for a in
            (CLOWa, CDIAGa, CLOWb, CDIAGb, WST, QSROW, IDN)]


def _split_waits(nc):
    """Hoist attached sync waits onto standalone EventSemaphore carriers.

    The walrus build in this container supports only one sync-wait command per
    instruction ("Too many sync wait commands") while Tile attaches all waits
    to the consuming instruction. A carrier EventSemaphore per wait, on the
    same engine right before the instruction, is semantically identical.
    """
    for bb in nc.main_func.blocks:
        insts = list(bb.instructions)
        out = []
        for ins in insts:
            si = getattr(ins, "sync_info", None)
            if si is not None and si.on_wait and type(ins).__name__ != "InstEventSemaphore":
                for k, w in enumerate(list(si.on_wait)):
                    ev = mybir.InstEventSemaphore(name=f"{ins.name}-sw{k}", ins=[], outs=[])
                    ev.engine = ins.engine
                    ev.sync_info = mybir.SyncInfo(on_wait=[w], on_update=[])
                    nc.register_instruction(ev)
                    out.append(ev)
                ins.sync_info = mybir.SyncInfo(on_wait=[], on_update=list(si.on_update))
            out.append(ins)
        bb.instructions[:] = out
    return nc


# fp8 phase-A load schedule, all on the SP HWDGE queue so the serial DMA
# engine processes transfers in exactly this order: "w" = one 128-col chunk of
# the QK weights (chunk-major, contiguous in DRAM so the descriptor overhead
# stays low), "x" = one 512-seq chunk of x, "v" = the V-column weights, and
# constants placed at the latest spot before their first consumer.
DMA_PLAN8 = [("xh", 0), ("w", 0), ("w", 1), ("w", 2), ("xh", 1),
             ("c", "qsrow"), ("w", 3), ("w", 4), ("w", 5), ("w", 6), ("w", 7),
             ("x", 1), ("c", "wst"), ("v", 0), ("x", 2),
             ("c", "clowa"), ("c", "cdiaga"), ("x", 3), ("c", "clowb"),
             ("c", "cdiagb")]
PROJ_TILES = [(0, 512), (512, 1024), (1024, 1536), (1536, 2048)]


def build_nc_fp8():
    CLOWa, CDIAGa, CLOWb, CDIAGb, WST, QSROW, IDN = _consts(pad_b=False)
    QSROW64 = (QSROW / WS).astype(np.float32)
    nc = bass.Bass()
    xcr_d = nc.declare_dram_parameter("xcr", [4, 128, NKD, 2, 512], F8,
                                      isOutput=False)
    # duplicate of x's first 256 seq columns, contiguous, so the very first
    # load is byte-bound (the strided half-chunk slice is descriptor-bound)
    x0a_d = nc.declare_dram_parameter("x0a", [128, NKD, 2, 256], F8,
                                      isOutput=False)
    wcr_d = nc.declare_dram_parameter("wcr", [8, 128, NKD, 2, 128], F8,
                                      isOutput=False)
    wv_d = nc.declare_dram_parameter("wv", [128, NKD, 2, H], F8, isOutput=False)
    out_d = nc.declare_dram_parameter("out", [S, H], F32, isOutput=True)

    clowa_d = nc.inline_tensor(CLOWa, "clowa")
    cdiaga_d = nc.inline_tensor(CDIAGa, "cdiaga")
    clowb_d = nc.inline_tensor(CLOWb, "clowb")
    cdiagb_d = nc.inline_tensor(CDIAGb, "cdiagb")
    wst_d = nc.inline_tensor(WST, "wst")
    qsrow_d = nc.inline_tensor(QSROW64, "qsrow")
    idn_d = nc.inline_tensor(IDN.astype(_bf), "idn")

    with tile.TileContext(nc) as tc:
        with (
            tc.tile_pool(name="singles", bufs=1) as singles,
            tc.tile_pool(name="bigbufs", bufs=1) as bigbufs,
            tc.tile_pool(name="spsum", bufs=1, space="PSUM") as spsum,
        ):

            # ---- constants (loaded via DMA_PLAN8 on the SP queue) ----
            qsrow = singles.tile([128, 512], F32)
            wst = singles.tile([128, NB], F32)
            clowa = singles.tile([L, 2 * L], F32)
            cdiaga = singles.tile([L, 2 * L], F32)
            clowb = singles.tile([L, L], F32)
            cdiagb = singles.tile([L, L], F32)
            const_map = {"qsrow": (qsrow, qsrow_d), "wst": (wst, wst_d),
                         "clowa": (clowa, clowa_d), "cdiaga": (cdiaga, cdiaga_d),
                         "clowb": (clowb, clowb_d),
                         "cdiagb": (cdiagb, cdiagb_d)}

            # ---- persistent SBUF operands (all bf16) ----
            qTb = bigbufs.tile([128, NKH, S], BF16, tag="qT")
            # block-major K^T so each block's [h, kt, pos] slice is contiguous
            # for the XBAR DMA transpose
            kTb = bigbufs.tile([128, NB, NKH, 128], BF16, tag="kT")
            vb = bigbufs.tile([128, NB, H], BF16, tag="v")
            kwb = bigbufs.tile([128, NB, H], BF16, tag="kw")
            stil = [spsum.tile([128, H], F32, tag=f"s{t}", name=f"stil{t}")
                    for t in range(NKH)]

            # ---- phase A: fp8 DoubleRow projections + K transposes ----
            with (
                tc.tile_pool(name="xw", bufs=1) as xw,
                tc.tile_pool(name="ppsum", bufs=4, space="PSUM") as rot,
            ):
                # PE warm-up: ramp the p-state while the first DMAs land.
                # 6 dummy 512-row matmuls keep the engine continuously busy
                # until the first projection matmul is ready, so real work
                # starts at the full 2.4 GHz p-state.
                wu = xw.tile([128, 512], BF16, tag="wu", name="wu")
                nc.gpsimd.memset(wu, 0.25)
                wupp = rot.tile([128, 512], F32, tag="pp", name="wupp")
                for i in range(6):
                    nc.tensor.matmul(wupp, lhsT=wu[:, 0:128], rhs=wu,
                                     start=(i == 0), stop=(i == 5),
                                     skip_group_check=True)

                xs = xw.tile([128, 4, NKD, 2, 512], F8, tag="xcr")
                ws = xw.tile([128, 8, NKD, 2, 128], F8, tag="wcr")
                wv = xw.tile([128, NKD, 2, H], F8, tag="wv")
                for kind, c in DMA_PLAN8:
                    if kind == "w":
                        nc.sync.dma_start(out=ws[:, c], in_=wcr_d[c])
                    elif kind == "xh":
                        if c == 0:
                            nc.sync.dma_start(out=xs[:, 0, :, :, 0:256],
                                              in_=x0a_d[:, :, :, :])
                        else:
                            nc.sync.dma_start(out=xs[:, 0, :, :, 256:512],
                                              in_=xcr_d[0][:, :, :, 256:512])
                    elif kind == "x":
                        nc.sync.dma_start(out=xs[:, c], in_=xcr_d[c])
                    elif kind == "v":
                        nc.sync.dma_start(out=wv, in_=wv_d[:, :, :, :])
                    else:
                        t, d = const_map[c]
                        nc.sync.dma_start(out=t, in_=d[:, :])

                # Q'^T and K^T tiles: [3h-tile, seq-tile]. The first seq
                # chunk runs as two 256-wide half-groups so matmuls start as
                # soon as the first half of x0 lands.
                for nt in range(4):
                    c0 = nt * 512
                    halves = ((0, 256), (256, 512)) if nt == 0 else ((0, 512),)
                    for mt in range(8):
                        pp = rot.tile([128, 512], F32, tag="pp", name="ppqk")
                        for h0, h1 in halves:
                            for j, kp in enumerate((0, 2)):
                                nc.tensor.matmul(
                                    pp[:, h0:h1],
                                    lhsT=ws[:, mt, kp:kp + 2, 0, :],
                                    rhs=xs[:, nt, kp:kp + 2, 1, h0:h1],
                                    start=(j == 0), stop=False,
                                    perf_mode=DR, skip_group_check=True)
                            for kt in range(NKD):
                                nc.tensor.matmul(
                                    pp[:, h0:h1], lhsT=ws[:, mt, kt, :, :],
                                    rhs=xs[:, nt, kt, :, h0:h1],
                                    start=False, stop=(kt == NKD - 1),
                                    perf_mode=DR, skip_group_check=True)
                        if mt < 4:
                            nc.vector.tensor_mul(
                                qTb[:, mt, c0:c0 + 512], pp, qsrow)
                        else:
                            nc.scalar.activation(
                                kTb[:, 4 * nt:4 * nt + 4, mt - 4, :], pp,
                                AF.Copy, scale=1.0 / WS)

                # V rows: [seq-block, h] — x stationary, W_v moving
                for I in range(NB):
                    ch, b = I // 4, I % 4
                    bcols = slice(b * 128, (b + 1) * 128)
                    pp = rot.tile([128, 512], F32, tag="pp", name="ppv")
                    for j, kp in enumerate((0, 2)):
                        nc.tensor.matmul(
                            pp, lhsT=xs[:, ch, kp:kp + 2, 1, bcols],
                            rhs=wv[:, kp:kp + 2, 0, :],
                            start=(j == 0), stop=False,
                            perf_mode=DR, skip_group_check=True)
                    for kt in range(NKD):
                        nc.tensor.matmul(
                            pp, lhsT=xs[:, ch, kt, :, bcols],
                            rhs=wv[:, kt, :, :],
                            start=False, stop=(kt == NKD - 1),
                            perf_mode=DR, skip_group_check=True)
                    nc.vector.tensor_scalar_mul(vb[:, I, :], pp, 1.0 / WS)

                # weighted K rows via XBAR DMA transpose (out[p,k,c] =
                # in[c,k,p], i.e. a per-kt-tile transpose in one DMA) +
                # scaled copy; frees the tensor engine entirely
                for I in range(NB):
                    kwt = xw.tile([128, NKH, 128], BF16, tag="kwt", name="kwt",
                                  bufs=3)
                    nc.sync.dma_start_transpose(out=kwt, in_=kTb[:, I])
                    nc.scalar.activation(
                        kwb[:, I, :], kwt, AF.Copy, scale=wst[:, I:I + 1])

            # ---- phase B: per-pair attention + state scan ----
            with (
                tc.tile_pool(name="blk", bufs=4) as blk,
                tc.tile_pool(name="bpsum", bufs=2, space="PSUM") as bpsum,
                tc.tile_pool(name="srd", bufs=3) as srd,
            ):
                def emit_pt(m):
                    s0 = bass.ts(2 * m, L)
                    s1 = bass.ts(2 * m + 1, L)
                    sq = bass.ds(2 * m * L, 2 * L)
                    ptab = bpsum.tile([L, 3 * L], F32, tag="pt", name="ptab")
                    pta = ptab[:, 0:2 * L]
                    ptb = ptab[:, 2 * L:3 * L]
                    for kt in range(NKH):
                        nc.tensor.matmul(pta, lhsT=kTb[:, 2 * m, kt, :],
                                         rhs=qTb[:, kt, sq],
                                         start=(kt == 0), stop=(kt == NKH - 1),
                                         skip_group_check=True)
                    for kt in range(NKH):
                        nc.tensor.matmul(ptb, lhsT=kTb[:, 2 * m + 1, kt, :],
                                         rhs=qTb[:, kt, s1],
                                         start=(kt == 0), stop=(kt == NKH - 1),
                                         skip_group_check=True)
                    return pta, ptb

                def emit_chain(m, pta, ptb):
                    aca = blk.tile([L, 2 * L], BF16, tag="aca", name="aca")
                    acb = blk.tile([L, L], BF16, tag="acb", name="acb")
                    for (pt, cd, cl, ac, w) in (
                            (pta, cdiaga, clowa, aca, 2 * L),
                            (ptb, cdiagb, clowb, acb, L)):
                        scratch = blk.tile([L, 2 * L], F32, tag="scratch",
                                           name="scratch")
                        nc.vector.tensor_mul(scratch[:, :w], pt[:, :w], cd[:, :w])
                        rs = blk.tile([L, 1], F32, tag="rs", name="rs")
                        nc.vector.reduce_sum(out=rs, in_=scratch[:, :w],
                                             axis=mybir.AxisListType.X)
                        # pcl is independent of the reduce -> off critical path
                        pcl = blk.tile([L, 2 * L], F32, tag="pcl", name="pcl")
                        nc.vector.tensor_mul(pcl[:, :w], pt[:, :w], cl[:, :w])
                        rcp = blk.tile([L, 1], F32, tag="rcp", name="rcp")
                        nc.scalar.activation(rcp, rs, AF.Abs)
                        nc.vector.tensor_scalar_max(rcp, rcp, 1.0)
                        nc.vector.reciprocal(rcp, rcp)
                        # ac = (pt*cd)*rcp + pt*cl fused
                        nc.vector.scalar_tensor_tensor(
                            ac[:, :w], scratch[:, :w], rcp, pcl[:, :w],
                            ALU.mult, ALU.add)
                    return aca, acb

                def emit_rb(m):
                    # state readback for pair m, issued right after pair m-1's
                    # state update so it overlaps pair m-1's output matmuls;
                    # kt3 on DVE, rest on Act
                    rb = float(GC ** (128.0 * m))
                    ssbs = []
                    for kt in range(NKH):
                        ssb = srd.tile([128, H], BF16, tag=f"ssb{kt}",
                                       name=f"ssb{kt}")
                        if kt == 3:
                            nc.vector.tensor_scalar_mul(ssb, stil[kt], rb)
                        else:
                            nc.scalar.activation(ssb, stil[kt], AF.Copy,
                                                 scale=rb)
                        ssbs.append(ssb)
                    return ssbs

                def emit_ops(m, aca, acb, ssbs):
                    s0 = bass.ts(2 * m, L)
                    s1 = bass.ts(2 * m + 1, L)
                    last = (m == NP - 1)
                    osb = blk.tile([L, 2, H], F32, tag="osb", name="osb")
                    # out(P0) = Aca[:, :128]^T V(P0) (+ Q'(P0) S)
                    op0 = bpsum.tile([L, H], F32, tag="out", name="op0")
                    nc.tensor.matmul(op0, lhsT=aca[:, 0:L], rhs=vb[:, 2 * m, :],
                                     start=True, stop=(m == 0))
                    if m > 0:
                        for kt in range(NKH):
                            nc.tensor.matmul(op0, lhsT=qTb[:, kt, s0],
                                             rhs=ssbs[kt],
                                             start=False, stop=(kt == NKH - 1))
                    # copies split Act/DVE so each psum bank frees in ~400ns;
                    # the last pair keeps everything on DVE + the SP queue so
                    # no cross-engine wait sits in front of the final DMAs
                    nc.scalar.activation(osb[:, 0, :], op0, AF.Copy)
                    if last:
                        nc.sync.dma_start(out=out_d[s0, :], in_=osb[:, 0, :])

                    # out(P1) = Aca[:, 128:]^T V(P0) + Acb^T V(P1) (+ Q'(P1) S)
                    op1 = bpsum.tile([L, H], F32, tag="out", name="op1")
                    nc.tensor.matmul(op1, lhsT=aca[:, L:2 * L],
                                     rhs=vb[:, 2 * m, :],
                                     start=True, stop=False)
                    nc.tensor.matmul(op1, lhsT=acb, rhs=vb[:, 2 * m + 1, :],
                                     start=False, stop=(m == 0))
                    if m > 0:
                        for kt in range(NKH):
                            nc.tensor.matmul(op1, lhsT=qTb[:, kt, s1],
                                             rhs=ssbs[kt],
                                             start=False, stop=(kt == NKH - 1))
                    if last:
                        nc.vector.tensor_copy(osb[:, 1, :], op1)
                        nc.sync.dma_start(out=out_d[s1, :], in_=osb[:, 1, :])
                    else:
                        nc.scalar.activation(osb[:, 1, :], op1, AF.Copy)
                        # one 256-row DMA per pair halves the out-DMA
                        # dispatches; rearrange makes the DRAM side iterate
                        # (pos, block, col) to match the SBUF tile layout
                        out_v = out_d.rearrange("(b l p) c -> p (b l) c",
                                                l=2, p=L)
                        nc.sync.dma_start(out=out_v[:, 2 * m:2 * m + 2, :],
                                          in_=osb)

                    # state update; the last pair's state is never read
                    if m < NP - 1:
                        for blki in (2 * m, 2 * m + 1):
                            for ht in range(NKH):
                                nc.tensor.matmul(
                                    stil[ht],
                                    lhsT=kwb[:, blki, ht * 128:(ht + 1) * 128],
                                    rhs=vb[:, blki, :],
                                    start=(m == 0 and blki == 0),
                                    stop=(m == NP - 2 and blki == 2 * m + 1),
                                    skip_group_check=True)
                        return emit_rb(m + 1)
                    return None

                # software pipeline: PT(m+1) is issued before ops(m) so the
                # tensor engine never stalls on pair m's DVE mask chain
                prev = None
                ssbs = None
                for m in range(NP):
                    pt = emit_pt(m)
                    ch = emit_chain(m, *pt)
                    if prev is not None:
                        ssbs = emit_ops(m - 1, *prev, ssbs)
                    prev = ch
                emit_ops(NP - 1, *prev, ssbs)
    return _split_waits(nc)


_NC_CACHE = {}

# test-harness knobs (the graded path leaves these at defaults)
TRACE = False
LAST_RESULT = None


def build_nc(has_bias: bool):
    return build_nc_fp8()


def _get_nc(key):
    if key not in _NC_CACHE:
        _NC_CACHE[key] = build_nc_fp8() if key == "fp8" else build_nc_f32r(True)
    return _NC_CACHE[key]


def _pack_levels(a):
    """[512, N] f32 -> hi/lo fp8 packed [128, NKD, 2, N] with lvl0=lo, lvl1=hi
    for x; the W packer flips levels (lvl0=hi, lvl1=lo)."""
    hi = a.astype(_f8)
    lo = (a - hi.astype(np.float32)).astype(_f8)
    return hi, lo


def kernel(x, W, b):
    global LAST_RESULT
    x = np.ascontiguousarray(x, dtype=np.float32)
    W = np.ascontiguousarray(W, dtype=np.float32)
    b = np.ascontiguousarray(b, dtype=np.float32)
    has_bias = bool(np.any(b))
    if has_bias:
        nc = _get_nc("f32r")
        in_maps = [
            {"xT": np.ascontiguousarray(x[i].T), "W": W, "b": b.reshape(1, 3 * H)}
            for i in range(B)
        ]
        res = run_bass_kernel_spmd(nc, in_maps, list(range(B)), trace=TRACE)
        LAST_RESULT = res
        return np.stack([res.results[i]["out"] for i in range(B)], axis=0)

    nc = _get_nc("fp8")
    Whi, Wlo = _pack_levels(W * WS)
    # QK weight chunks: [chunk, part, kt, lvl, col] with lvl0=hi, lvl1=lo
    wcr = np.empty((8, 128, NKD, 2, 128), _f8)
    wv = np.empty((128, NKD, 2, H), _f8)
    for lvl, Wq in ((0, Whi), (1, Wlo)):
        r = Wq.reshape(NKD, 128, 3 * H).transpose(1, 0, 2)  # [part, kt, col]
        for c in range(8):
            wcr[c, :, :, lvl, :] = r[:, :, c * 128:(c + 1) * 128]
        wv[:, :, lvl, :] = r[:, :, 2 * H:]
    in_maps = []
    for i in range(B):
        xT = np.ascontiguousarray(x[i].T)
        xhi, xlo = _pack_levels(xT)
        # x chunks: [chunk, part, kt, lvl, seq] with lvl0=lo, lvl1=hi
        xcr = np.empty((4, 128, NKD, 2, 512), _f8)
        for lvl, xq in ((0, xlo), (1, xhi)):
            r = xq.reshape(NKD, 128, S).transpose(1, 0, 2)
            for c in range(4):
                xcr[c, :, :, lvl, :] = r[:, :, c * 512:(c + 1) * 512]
        x0a = np.ascontiguousarray(xcr[0, :, :, :, 0:256])
        in_maps.append({"xcr": xcr, "wcr": wcr, "wv": wv, "x0a": x0a})
    res = run_bass_kernel_spmd(nc, in_maps, list(range(B)), trace=TRACE)
    LAST_RESULT = res
    return np.stack([res.results[i]["out"] for i in range(B)], axis=0)


# ---------------------------------------------------------------------------
# f32r fallback (bias path; mirrors the validated baseline kernel)
# ---------------------------------------------------------------------------

DMA_PLAN = [("w", 0, 512), ("x", 0, 512), ("w", 512, 1024), ("x", 512, 1024),
            ("x", 1024, 1536), ("x", 1536, 2048), ("w", 1024, 1536)]


def build_nc_f32r(has_bias: bool, mmdt=mybir.dt.float32r):
    CLOWa, CDIAGa, CLOWb, CDIAGb, WST, QSROW, IDN = _consts(pad_b=True)
    QSROW = np.tile(QSROW[:1], (128, 1)).astype(np.float32)
    nc = bass.Bass()
    xT_d = nc.declare_dram_parameter("xT", [D, S], mmdt, isOutput=False)
    w_d = nc.declare_dram_parameter("W", [D, 3 * H], mmdt, isOutput=False)
    b_d = nc.declare_dram_parameter("b", [1, 3 * H], mmdt, isOutput=False)
    out_d = nc.declare_dram_parameter("out", [S, H], F32, isOutput=True)

    clowa_d = nc.inline_tensor(CLOWa, "clowa")
    cdiaga_d = nc.inline_tensor(CDIAGa, "cdiaga")
    clowb_d = nc.inline_tensor(CLOWb, "clowb")
    cdiagb_d = nc.inline_tensor(CDIAGb, "cdiagb")
    wst_d = nc.inline_tensor(WST, "wst")
    qsrow_d = nc.inline_tensor(QSROW, "qsrow")
    idn_d = nc.inline_tensor(IDN, "idn")

    with tile.TileContext(nc) as tc:
        with (
            tc.tile_pool(name="singles", bufs=1) as singles,
            tc.tile_pool(name="bigbufs", bufs=1) as bigbufs,
            tc.tile_pool(name="spsum", bufs=1, space="PSUM") as spsum,
        ):
            clowa = singles.tile([L, 2 * L], F32)
            nc.gpsimd.dma_start(out=clowa, in_=clowa_d[:, :])
            cdiaga = singles.tile([L, 2 * L], F32)
            nc.gpsimd.dma_start(out=cdiaga, in_=cdiaga_d[:, :])
            clowb = singles.tile([L, 2 * L], F32)
            nc.gpsimd.dma_start(out=clowb, in_=clowb_d[:, :])
            cdiagb = singles.tile([L, 2 * L], F32)
            nc.gpsimd.dma_start(out=cdiagb, in_=cdiagb_d[:, :])
            wst = singles.tile([128, NB], F32)
            nc.gpsimd.dma_start(out=wst, in_=wst_d[:, :])
            qsrow = singles.tile([128, 512], F32)
            nc.gpsimd.dma_start(out=qsrow, in_=qsrow_d[:, :])
            idn = singles.tile([128, 128], mmdt)
            nc.gpsimd.dma_start(out=idn, in_=idn_d[:, :])

            qT = bigbufs.tile([128, NKH, S], mmdt, tag="qT")
            kT = bigbufs.tile([128, NKH, S], mmdt, tag="kT")
            v_all = bigbufs.tile([128, NB, H], mmdt, tag="v")
            kw_all = bigbufs.tile([128, NB, H], mmdt, tag="kw")
            stil = [spsum.tile([128, H], F32, tag=f"s{t}", name=f"stil{t}")
                    for t in range(NKH)]

            with (
                tc.tile_pool(name="xw", bufs=1) as xw,
                tc.tile_pool(name="ppsum", bufs=4, space="PSUM") as ppsum,
            ):
                xT_r = xT_d.rearrange("(k p) s -> p k s", p=128)
                w_r = w_d.rearrange("(k p) m -> p k m", p=128)
                xTs = xw.tile([128, NKD, S], mmdt, tag="xT")
                ws = xw.tile([128, NKD, 3 * H], mmdt, tag="w")

                def ldw(c0, c1):
                    for kt in range(NKD):
                        nc.sync.dma_start(out=ws[:, kt, c0:c1], in_=w_r[:, kt, c0:c1])

                def ldx(c0, c1):
                    for kt in range(NKD):
                        nc.sync.dma_start(out=xTs[:, kt, c0:c1], in_=xT_r[:, kt, c0:c1])

                for (kind, c0, c1) in DMA_PLAN:
                    (ldw if kind == "w" else ldx)(c0, c1)
                if has_bias:
                    brow = xw.tile([1, 3 * H], mmdt, tag="b")
                    nc.sync.dma_start(out=brow, in_=b_d[:, :])
                    ones = xw.tile([1, S], mmdt, tag="ones")
                    nc.vector.memset(ones, 1.0)

                for nt, (c0, c1) in enumerate(PROJ_TILES):
                    w_ = c1 - c0
                    for mt in range(8):
                        pp = ppsum.tile([128, 512], F32, tag="pp")
                        for kt in range(NKD):
                            nc.tensor.matmul(
                                pp[:, :w_], lhsT=ws[:, kt, mt * 128:(mt + 1) * 128],
                                rhs=xTs[:, kt, c0:c1],
                                start=(kt == 0), stop=(kt == NKD - 1 and not has_bias),
                                skip_group_check=True)
                        if has_bias:
                            nc.tensor.matmul(
                                pp[:, :w_], lhsT=brow[:, mt * 128:(mt + 1) * 128],
                                rhs=ones[:, c0:c1],
                                start=False, stop=True, skip_group_check=True)
                        if mt < 4:
                            nc.vector.tensor_mul(
                                qT[:, mt, c0:c1], pp[:, :w_], qsrow[:, :w_])
                        else:
                            nc.scalar.activation(
                                kT[:, mt - 4, c0:c1], pp[:, :w_], AF.Copy)

                for I in range(NB):
                    pp = ppsum.tile([128, 512], F32, tag="pp")
                    for kt in range(NKD):
                        nc.tensor.matmul(
                            pp, lhsT=xTs[:, kt, I * 128:(I + 1) * 128],
                            rhs=ws[:, kt, 2 * H:3 * H],
                            start=(kt == 0), stop=(kt == NKD - 1 and not has_bias))
                    if has_bias:
                        nc.tensor.matmul(
                            pp, lhsT=ones[:, I * 128:(I + 1) * 128],
                            rhs=brow[:, 2 * H:3 * H], start=False, stop=True)
                    nc.scalar.activation(v_all[:, I, :], pp, AF.Copy)

                for I in range(NB):
                    pp = ppsum.tile([128, 512], mmdt, tag="pp")
                    for kt in range(NKH):
                        nc.tensor.matmul(
                            pp[:, kt * 128:(kt + 1) * 128],
                            lhsT=kT[:, kt, I * 128:(I + 1) * 128], rhs=idn,
                            is_transpose=True, start=True, stop=True,
                            skip_group_check=True)
                    nc.scalar.activation(
                        kw_all[:, I, :], pp, AF.Copy, scale=wst[:, I:I + 1])

            with (
                tc.tile_pool(name="blk", bufs=4) as blk,
                tc.tile_pool(name="bpsum", bufs=2, space="PSUM") as bpsum,
                tc.tile_pool(name="srd", bufs=3) as srd,
            ):
                for m in range(NP):
                    s0 = bass.ts(2 * m, L)
                    s1 = bass.ts(2 * m + 1, L)
                    sq = bass.ds(2 * m * L, 2 * L)
                    wb = 2 * L if m < NP - 1 else L
                    sb = bass.ds((2 * m + 1) * L, wb)
                    ptab = bpsum.tile([L, 4 * L], F32, tag="pt")
                    pta = ptab[:, 0:2 * L]
                    ptb = ptab[:, 2 * L:3 * L]
                    for kt in range(NKH):
                        nc.tensor.matmul(pta, lhsT=kT[:, kt, s0], rhs=qT[:, kt, sq],
                                         start=(kt == 0), stop=(kt == NKH - 1),
                                         skip_group_check=True)
                    for kt in range(NKH):
                        nc.tensor.matmul(ptab[:, 2 * L:2 * L + wb],
                                         lhsT=kT[:, kt, s1], rhs=qT[:, kt, sb],
                                         start=(kt == 0), stop=(kt == NKH - 1),
                                         skip_group_check=True)

                    aca = blk.tile([L, 2 * L], mmdt, tag="aca")
                    acb = blk.tile([L, L], mmdt, tag="acb")
                    for (pt, cd, cl, ac, w) in ((pta, cdiaga, clowa, aca, 2 * L),
                                                (ptb, cdiagb, clowb, acb, L)):
                        scratch = blk.tile([L, 2 * L], F32, tag="scratch")
                        nc.vector.tensor_mul(scratch[:, :w], pt[:, :w], cd[:, :w])
                        rs = blk.tile([L, 1], F32, tag="rs")
                        nc.vector.reduce_sum(out=rs, in_=scratch[:, :w],
                                             axis=mybir.AxisListType.X)
                        rcp = blk.tile([L, 1], F32, tag="rcp")
                        nc.scalar.activation(rcp, rs, AF.Abs)
                        nc.vector.tensor_scalar_max(rcp, rcp, 1.0)
                        nc.vector.reciprocal(rcp, rcp)
                        pcl = blk.tile([L, 2 * L], F32, tag="pcl")
                        nc.vector.tensor_mul(pcl[:, :w], pt[:, :w], cl[:, :w])
                        nc.vector.scalar_tensor_tensor(
                            ac[:, :w], scratch[:, :w], rcp, pcl[:, :w],
                            ALU.mult, ALU.add)

                    if m > 0:
                        rb = float(GC ** (128.0 * m))
                        ssbs = []
                        for kt in range(NKH):
                            ssb = srd.tile([128, H], mmdt, tag=f"ssb{kt}",
                                           name=f"ssb{kt}")
                            nc.scalar.activation(ssb, stil[kt], AF.Copy, scale=rb)
                            ssbs.append(ssb)

                    op0 = bpsum.tile([L, H], F32, tag="out")
                    nc.tensor.matmul(op0, lhsT=aca[:, 0:L], rhs=v_all[:, 2 * m, :],
                                     start=True, stop=(m == 0))
                    if m > 0:
                        for kt in range(NKH):
                            nc.tensor.matmul(op0, lhsT=qT[:, kt, s0], rhs=ssbs[kt],
                                             start=False, stop=(kt == NKH - 1))
                    osb0 = blk.tile([L, H], F32, tag="osb0")
                    nc.vector.tensor_copy(osb0, op0)
                    nc.sync.dma_start(out=out_d[s0, :], in_=osb0)

                    op1 = bpsum.tile([L, H], F32, tag="out")
                    nc.tensor.matmul(op1, lhsT=aca[:, L:2 * L], rhs=v_all[:, 2 * m, :],
                                     start=True, stop=False)
                    nc.tensor.matmul(op1, lhsT=acb, rhs=v_all[:, 2 * m + 1, :],
                                     start=False, stop=(m == 0))
                    if m > 0:
                        for kt in range(NKH):
                            nc.tensor.matmul(op1, lhsT=qT[:, kt, s1], rhs=ssbs[kt],
                                             start=False, stop=(kt == NKH - 1))
                    osb1 = blk.tile([L, H], F32, tag="osb1")
                    nc.vector.tensor_copy(osb1, op1)
                    nc.sync.dma_start(out=out_d[s1, :], in_=osb1)

                    for blki in (2 * m, 2 * m + 1):
                        for ht in range(NKH):
                            nc.tensor.matmul(
                                stil[ht],
                                lhsT=kw_all[:, blki, ht * 128:(ht + 1) * 128],
                                rhs=v_all[:, blki, :],
                                start=(m == 0 and blki == 0), stop=(m == NP - 1),
                                skip_group_check=True)
    return _split_waits(nc)


if __name__ == "__main__":
    xs = np.random.randn(B, S, D).astype(np.float32)
    Ws = (np.random.randn(D, 3 * H) * 0.02).astype(np.float32)
    bs = np.zeros(3 * H, np.float32)
    out = kernel(xs, Ws, bs)
    print(out.shape, out.dtype)
